# revision 1
# baseline (speedup 1.0000x reference)
"""CapsuleLayer (single routing iteration) Trainium2 kernel.

Math (per batch element b of x: (B=64, NU=32, IC=256, US=128) fp32):
  - torch-style reshape of x[b] to (IC, NU, US): row i of the flat
    (256, 4096) view is x[b].flat[i*4096:(i+1)*4096].
  - s[j]   = (1/256) * sum_i flat[i, j]          (j = n*128+u, 4096 outputs)
  - msq[n] = sum_u s[n,u]^2
  - out[n,u] = msq/(1+msq) * s[n,u]/(sqrt(msq)+1e-5)

Sharding: pure batch data-parallel over 8 NeuronCores (8 batches/core).

Per-core pipeline (memory-bound; ~32 MiB HBM reads per core):
  - 2x 2MiB HWDGE loads per batch -> t (128, 2, 4096) SBUF.
  - 16 float32r matmuls (1 cycle/row) reduce ic over both partition-halves
    into one PSUM (8, 512) bank accumulation group; lhsT column k holds
    1/256 so chunk k's sums land in PSUM partition k.
  - ScalarE Square+accum -> msq, Sqrt -> mag; VectorE builds
    factor = msq / ((1+msq)(mag+1e-5)); out = s * factor; DMA out.

float32r streams fp32 through the PE in one pass by rounding the operands
to ~12 mantissa bits (same 4-byte encoding, so the host feeds plain fp32
bytes); weights are powers of two (exact), so output rel-err is ~1e-4
instead of fp32's ~1e-7, while PE time drops 4x vs the two-pass fp32 path.
MODE = "fp32" keeps the bit-exact variant (~147 us vs ~104 us).
"""

import numpy as np

import concourse.bass as bass
import concourse.bacc as bacc
import concourse.mybir as mybir
import concourse.tile as tile
from concourse.bass_utils import run_bass_kernel_spmd

B, NU, IC, US = 64, 32, 256, 128
N_CORES = 8
PB = B // N_CORES            # batches per core
F = NU * US                  # 4096 outputs per batch
HALVES = IC // 128           # 2 partition-halves of the ic axis
NBANK = F // 512             # 8 matmul chunks (one PSUM bank row each)
NQ = F // NBANK // 128       # 4 u-groups per PSUM partition row

# "fp32r": single-pass rounded-fp32 matmuls, all-HWDGE loads (fast path).
# "fp32": exact fp32 matmuls + SWDGE DMA-accumulate ic-halves (slow, exact).
MODE = "fp32r"


def build_bass(pb=PB, mode=MODE):
    PB = pb
    nc = bacc.Bacc("TRN2", target_bir_lowering=False, debug=False)

    mm_dt = mybir.dt.float32r if mode == "fp32r" else mybir.dt.float32

    # float32r shares the fp32 byte encoding (it is fp32 with the mantissa
    # rounded to ~12 bits by the PE), so the host feeds plain fp32 bytes.
    x = nc.dram_tensor("x", [PB, HALVES, 128, F], mm_dt,
                       kind="ExternalInput")
    w = nc.dram_tensor("w", [128, NBANK, NBANK], mm_dt,
                       kind="ExternalInput")
    y = nc.dram_tensor("y", [PB, NBANK, 512], mybir.dt.float32,
                       kind="ExternalOutput")

    with tile.TileContext(nc) as tc:
        with (
            tc.tile_pool(name="const", bufs=1) as const_pool,
            tc.tile_pool(name="acc", bufs=11 if mode == "fp32r" else 8) as acc_pool,
            tc.tile_pool(name="psum", bufs=8, space="PSUM") as psum_pool,
            tc.tile_pool(name="scratch", bufs=2) as scratch_pool,
            tc.tile_pool(name="stats", bufs=12) as stats_pool,
            tc.tile_pool(name="outp", bufs=2) as out_pool,
        ):
            # Selection weights: sel[:, k, j] = 1/256 iff j == k.
            # (loaded from DRAM — memset can't emit float32r)
            sel = const_pool.tile([128, NBANK, NBANK], mm_dt)
            nc.sync.dma_start(out=sel[:], in_=w[:])

            def squash_tail(b, ps):
                # msq[c, q] = sum_u s^2 over each 128-wide u-group.
                msq = stats_pool.tile([NBANK, NQ], mybir.dt.float32, tag="msq")
                sq = scratch_pool.tile([NBANK, 512], mybir.dt.float32, tag="sq")
                for q in range(NQ):
                    nc.scalar.activation(
                        out=sq[:, q * 128 : (q + 1) * 128],
                        in_=ps[:, q * 128 : (q + 1) * 128],
                        func=mybir.ActivationFunctionType.Square,
                        accum_out=msq[:, q : q + 1],
                    )

                mag = stats_pool.tile([NBANK, NQ], mybir.dt.float32, tag="mag")
                nc.scalar.activation(out=mag[:], in_=msq[:],
                                     func=mybir.ActivationFunctionType.Sqrt)

                # factor = msq / ((1 + msq) * (mag + 1e-5))
                t2 = stats_pool.tile([NBANK, NQ], mybir.dt.float32, tag="t2")
                nc.vector.tensor_scalar_add(t2[:], mag[:], 1e-5)
                den = stats_pool.tile([NBANK, NQ], mybir.dt.float32, tag="den")
                nc.vector.scalar_tensor_tensor(
                    out=den[:], in0=msq[:], scalar=1.0, in1=t2[:],
                    op0=mybir.AluOpType.add, op1=mybir.AluOpType.mult)
                rec = stats_pool.tile([NBANK, NQ], mybir.dt.float32, tag="rec")
                nc.vector.reciprocal(rec[:], den[:])
                fac = stats_pool.tile([NBANK, NQ], mybir.dt.float32, tag="fac")
                nc.vector.tensor_mul(fac[:], msq[:], rec[:])

                outt = out_pool.tile([NBANK, 512], mybir.dt.float32, tag="outt")
                fap = fac[:]
                fac_bcast = bass.AP(tensor=fap.tensor, offset=fap.offset,
                                    ap=[fap.ap[0], fap.ap[1], [0, 128]])
                nc.vector.tensor_mul(
                    outt[:].rearrange("p (q u) -> p q u", q=NQ),
                    ps[:].rearrange("p (q u) -> p q u", q=NQ),
                    fac_bcast)

                # SWDGE (gpsimd) is otherwise idle in fp32r mode — keep the
                # small result stores off the load rings.
                if mode == "fp32r":
                    nc.gpsimd.dma_start(out=y[b], in_=outt[:])
                else:
                    nc.sync.dma_start(out=y[b], in_=outt[:])

            for b in range(PB):
                ps = psum_pool.tile([NBANK, 512], mybir.dt.float32, tag="ps")

                if mode == "fp32r":
                    # One (128, 4096) 2 MiB tile per ic-half; alternate the
                    # two HWDGE rings (SP via nc.sync, ACT via nc.scalar) so
                    # per-DMA completion bubbles overlap across rings.
                    for h in range(HALVES):
                        t = acc_pool.tile([128, F], mm_dt, tag="acc")
                        eng = nc.sync if h == 0 else nc.scalar
                        if b == 0 or (b == PB - 1 and h == HALVES - 1):
                            # Split the first loads (PE starts after ~1 MiB
                            # instead of ~2 MiB) and the last one (the final
                            # matmul burst trails the last bytes closely).
                            eng.dma_start(out=t[:, : F // 2],
                                          in_=x[b, h, :, : F // 2])
                            eng.dma_start(out=t[:, F // 2 :],
                                          in_=x[b, h, :, F // 2 :])
                        else:
                            eng.dma_start(out=t[:], in_=x[b, h])
                        for k in range(NBANK):
                            nc.tensor.matmul(
                                ps[:, :],
                                sel[:, k, :],
                                t[:, k * 512 : (k + 1) * 512],
                                start=(h == 0 and k == 0),
                                stop=(h == HALVES - 1 and k == NBANK - 1),
                            )
                else:
                    acc = acc_pool.tile([128, F], mybir.dt.float32, tag="acc")
                    nc.sync.dma_start(out=acc[:], in_=x[b, 0])
                    # CCE (DMA-inline add) descriptors are per partition row
                    # and max out at 2048 elements — split the accumulate.
                    for j in range(F // 2048):
                        nc.gpsimd.dma_start(
                            out=acc[:, j * 2048 : (j + 1) * 2048],
                            in_=x[b, 1, :, j * 2048 : (j + 1) * 2048],
                            accum_op=mybir.AluOpType.add)
                    for k in range(NBANK):
                        nc.tensor.matmul(
                            ps[:, :],
                            sel[:, k, :],
                            acc[:, k * 512 : (k + 1) * 512],
                            start=(k == 0),
                            stop=(k == NBANK - 1),
                        )

                squash_tail(b, ps)

    nc.compile()
    return nc


_NC_CACHE = {}


def _get_nc():
    if "nc" not in _NC_CACHE:
        _NC_CACHE["nc"] = build_bass()
    return _NC_CACHE["nc"]


def kernel(x, **run_kwargs):
    x = np.ascontiguousarray(np.asarray(x, dtype=np.float32))
    assert x.shape == (B, NU, IC, US), x.shape

    nc = _get_nc()
    xs = x.reshape(N_CORES, PB, HALVES, 128, F)
    w = np.zeros((128, NBANK, NBANK), dtype=np.float32)
    for k in range(NBANK):
        w[:, k, k] = 1.0 / IC
    in_maps = [{"x": np.ascontiguousarray(xs[c]), "w": w}
               for c in range(N_CORES)]
    res = run_bass_kernel_spmd(nc, in_maps, core_ids=list(range(N_CORES)),
                               **run_kwargs)
    out = np.stack([r["y"] for r in res.results], axis=0)  # (8, PB, 8, 512)
    out = out.reshape(B, NU, US, 1)
    if run_kwargs:
        kernel.last_results = res
    return out

